# revision 13
# baseline (speedup 1.0000x reference)
"""Multi-head self-attention on 8 TRN2 NeuronCores.

Problem: x(4,2048,1024), Wq(8,1024,128), Wk/Wv(1024,128), Wo(1024,1024) fp32.
out = softmax(Q K^T / sqrt(128)) V -> concat heads -> @ Wo.

Sharding: (batch, query-half) across 8 cores - core c handles batch c//2,
query rows [(c%2)*1024, (c%2)*1024+1024). K/V cover the full sequence of the
batch, so each core computes them locally from its x slice; no collectives.

Dataflow (v2): q-tile OUTER, head INNER so the out-projection (which needs all
heads of a q-tile) pipelines into the main loop instead of being a serial
tail. Per (qt, h) step:
  PE   : scores s0|s1 (f32r, 4x512-col) + lagged ctx (16 fp16 128-col mms
         against v_sb) + lagged out-projection mms (fp16 512-col)
  DVE  : reduce_max(negate) per score half, min-combine -> -gmax, reciprocal
  ACT  : exp per half (bias=-gmax, fp16 out, accum_out -> den halves),
         ctx-ring batch copies, out copies, small normalize slice
  Pool : den = den0+den1, bulk of the normalize (tensor_scalar by 1/den)
  DMA  : P^T via the xbar dma_start_transpose ([16,128]-tile hw transpose,
         SBUF->SBUF fp16) - replaces the PE transposes AND the PSUM->SBUF
         wave copies of the v1 kernel, freeing ~1.7us/step of PE+DVE time.
The normalization is a per-partition tensor_scalar on the q-part P (q on
partitions), which kills v1's replicate+PE-transpose+broadcast machinery.

PSUM (8 banks): 3x scores [128,1024] (6) + ctx ring [128,4,128] (1 bank,
slot=h%4) + out ring [128,512] (1 bank, ec-serialized accumulation over h).

Numerics: scores need ~12+ mantissa bits -> f32r end-to-end for x/Wq/Wk
(1 cycle/row at >=256-wide moving). P/V/ctx/Wo run fp16. exp against the
global row max: P in (0,1], den ~ O(1) in fp32 accumulators.
"""
import numpy as np

B, S, E, H, O = 4, 2048, 1024, 8, 128
SQ = S // 2          # query rows per core
NCORES = 8
ET = E // 128        # 8 e-tiles
ST = S // 128        # 16 s-tiles
QT = SQ // 128       # 8 q-tiles
HS = S // 2          # 1024-wide score half

LAG = 5              # steps between scores(t) and ctx(t) consumption
NORM_ACT = 256       # normalize slice handled by ACT (rest: Pool)

_compiled = None     # cache so repeated kernel() calls skip rebuild


def _build():
    import concourse.bass as bass
    import concourse.mybir as mybir
    import concourse.tile as tile
    from concourse import bacc

    F32 = mybir.dt.float32
    F32R = mybir.dt.float32r
    FP16 = mybir.dt.float16
    PS = bass.MemorySpace.PSUM
    EXP = mybir.ActivationFunctionType.Exp
    MIN = mybir.AluOpType.min
    ADD = mybir.AluOpType.add
    MUL = mybir.AluOpType.mult
    X = mybir.AxisListType.X

    nc = bacc.Bacc("TRN2", target_bir_lowering=False, debug=False,
                   enable_asserts=True)

    # xkv columns pre-permuted per core so its query half is always columns
    # [0, SQ) - attention is permutation-invariant over the key axis.
    d_xkv = nc.dram_tensor("xkv", (E, S), F32R, kind="ExternalInput").ap()
    d_wq = nc.dram_tensor("wq", (H, 128, ET * O), F32R, kind="ExternalInput").ap()
    d_wk = nc.dram_tensor("wk", (128, ET * O), F32R, kind="ExternalInput").ap()
    d_wv = nc.dram_tensor("wv", (128, ET * O), F32R, kind="ExternalInput").ap()
    d_wo = nc.dram_tensor("wo", (128, H * E), FP16, kind="ExternalInput").ap()
    d_out = nc.dram_tensor("out", (SQ, E), F32, kind="ExternalOutput").ap()

    with tile.TileContext(nc) as tc:
        with (
            tc.tile_pool(name="persist", bufs=1) as persist,
            tc.tile_pool(name="tiny", bufs=24) as tiny,
        ):
            wo_sb = persist.tile([128, H, E], FP16, tag="wo")
            kt = persist.tile([128, S], F32R, tag="kt")
            qt_sb = persist.tile([128, H, SQ], F32R, tag="qt")
            v_sb = persist.tile([128, ST, O], FP16, tag="v")

            # ---------------- prologue: K^T, V, Q^T projections ----------
            with tc.tile_pool(name="xp", bufs=1) as xp:
                wk = xp.tile([128, ET, O], F32R, tag="wk")
                wv = xp.tile([128, ET, O], F32R, tag="wv")
                xkv = xp.tile([128, ET, S], F32R, tag="xkv")
                wq = xp.tile([128, H, ET, O], F32R, tag="wq")
                vt_sb = xp.tile([128, S], FP16, tag="vtsb")

                # x e-tiles split over the two HWDGE queues; weights on the
                # gpsimd SWDGE queue so x saturates the aggregate early.
                for e in range(0, ET, 2):
                    nc.sync.dma_start(
                        xkv[:, e, :], d_xkv[e * 128:(e + 1) * 128, :])
                    nc.scalar.dma_start(
                        xkv[:, e + 1, :],
                        d_xkv[(e + 1) * 128:(e + 2) * 128, :])
                nc.sync.dma_start(wk[:].rearrange("p t o -> p (t o)"), d_wk)
                nc.scalar.dma_start(wv[:].rearrange("p t o -> p (t o)"), d_wv)
                for h in range(H):
                    nc.gpsimd.dma_start(
                        wq[:, h, :, :].rearrange("p t o -> p (t o)"), d_wq[h])
                nc.gpsimd.dma_start(
                    wo_sb[:].rearrange("p h e -> p (h e)"), d_wo)

                # K^T and V^T share one PSUM scope (4+4 banks); e-major so
                # the PE starts on the first arriving e-tile.
                with (
                    tc.tile_pool(name="ktp", bufs=1, space=PS) as ktp,
                    tc.tile_pool(name="vtp", bufs=1, space=PS) as vtp,
                ):
                    kt_ps = ktp.tile([128, S], F32, tag="kt")
                    vt_ps = vtp.tile([128, S], F32, tag="vt")
                    for e in range(ET):
                        for c in range(4):
                            nc.tensor.matmul(
                                kt_ps[:, c * 512:(c + 1) * 512],
                                wk[:, e, :],
                                xkv[:, e, c * 512:(c + 1) * 512],
                                start=(e == 0), stop=(e == ET - 1),
                            )
                        for c in range(4):
                            nc.tensor.matmul(
                                vt_ps[:, c * 512:(c + 1) * 512],
                                wv[:, e, :],
                                xkv[:, e, c * 512:(c + 1) * 512],
                                start=(e == 0), stop=(e == ET - 1),
                            )
                    nc.scalar.copy(kt[:], kt_ps[:])
                    nc.scalar.copy(vt_sb[:], vt_ps[:])
                # V^T (O,S) -> v_sb[kp, st, o] = V[st*128+kp, o] via the
                # hw xbar transpose (replaces 16 PE transposes + copies).
                ord_scr = xp.tile([128, 2], FP16, tag="ordscr")
                nc.sync.dma_start(ord_scr[:], vt_sb[:, 0:2])
                nc.sync.dma_start_transpose(v_sb[:], vt_sb[:])

                # Q^T per head (gated on wq[h] + all of x)
                with tc.tile_pool(name="qp", bufs=3, space=PS) as qp:
                    for h in range(H):
                        q_ps = qp.tile([128, SQ], F32, tag="qtps")
                        for e in range(ET):
                            for c in range(SQ // 512):
                                nc.tensor.matmul(
                                    q_ps[:, c * 512:(c + 1) * 512],
                                    wq[:, h, e, :],
                                    xkv[:, e, c * 512:(c + 1) * 512],
                                    start=(e == 0), stop=(e == ET - 1),
                                )
                        nc.scalar.copy(qt_sb[:, h, :], q_ps[:])

            # ---------------- main: qt outer, head inner ------------------
            with (
                tc.tile_pool(name="p_pool", bufs=4) as p_pool,
                tc.tile_pool(name="pn_pool", bufs=4) as pn_pool,
                tc.tile_pool(name="pt_pool", bufs=LAG + 2) as pt_pool,
                tc.tile_pool(name="cx_pool", bufs=3) as cx_pool,
                tc.tile_pool(name="o_sbp", bufs=2) as o_sbp,
                tc.tile_pool(name="sc_ps", bufs=3, space=PS) as sc_psp,
                tc.tile_pool(name="ct_ps", bufs=1, space=PS) as ct_psp,
                tc.tile_pool(name="o_ps", bufs=1, space=PS) as o_psp,
            ):
                ct_ring = ct_psp.tile([128, 4, 128], F32, tag="ct")
                o_ring = o_psp.tile([128, 512], F32, tag="op")

                NT = H * QT
                state = {}   # per-step tiles: s0, s1, p, pn, pt, nm2, den2..
                ctxs = {}    # qt -> ctxs_sb tile [128, H, 128]

                def emit_scores_half(t, sh):
                    qt, h = divmod(t, H)
                    st_ = state[t]
                    s_ps = sc_psp.tile([128, HS], F32, tag="sc")
                    st_["s%d" % sh] = s_ps
                    for c in range(2):
                        nc.tensor.matmul(
                            s_ps[:, c * 512:(c + 1) * 512],
                            qt_sb[:, h, qt * 128:(qt + 1) * 128],
                            kt[:, sh * HS + c * 512:sh * HS + (c + 1) * 512],
                            start=True, stop=True,
                        )

                def emit_ctx(t):
                    # ctx^T(o-part, 128 q cols) for step t, into ring slot
                    qt, h = divmod(t, H)
                    st_ = state[t]
                    slot = h % 4
                    for st in range(ST):
                        nc.tensor.matmul(
                            ct_ring[:, slot, :],
                            v_sb[:, st, :],
                            st_["pt"][:, st, :],
                            start=(st == 0), stop=(st == ST - 1),
                        )
                    if slot == 3:
                        # batch-copy 4 head slots into the qt's ctx tile
                        if qt not in ctxs:
                            ctxs[qt] = cx_pool.tile([128, H, 128], FP16,
                                                    tag="cx",
                                                    name="cx%d" % qt)
                        nc.scalar.copy(
                            ctxs[qt][:, h - 3:h + 1, :], ct_ring[:])
                    st_["pt"] = None  # allow pool reuse

                def emit_out(qt, part):
                    # out-projection for q-tile qt; part 0: heads 0-3 of the
                    # ec0 group; part 1: heads 4-7 + full ec1 group + copies
                    cx = ctxs[qt]
                    if part == 0:
                        for h in range(4):
                            nc.tensor.matmul(
                                o_ring[:], cx[:, h, :],
                                wo_sb[:, h, 0:512],
                                start=(h == 0), stop=False,
                            )
                        return
                    for h in range(4, H):
                        nc.tensor.matmul(
                            o_ring[:], cx[:, h, :],
                            wo_sb[:, h, 0:512],
                            start=False, stop=(h == H - 1),
                        )
                    o_sb = o_sbp.tile([128, E], F32, tag="osb")
                    nc.scalar.copy(o_sb[:, 0:512], o_ring[:])
                    nc.sync.dma_start(
                        d_out[qt * 128:(qt + 1) * 128, 0:512],
                        o_sb[:, 0:512])
                    for h in range(H):
                        nc.tensor.matmul(
                            o_ring[:], cx[:, h, :],
                            wo_sb[:, h, 512:1024],
                            start=(h == 0), stop=(h == H - 1),
                        )
                    nc.scalar.copy(o_sb[:, 512:1024], o_ring[:])
                    nc.sync.dma_start(
                        d_out[qt * 128:(qt + 1) * 128, 512:1024],
                        o_sb[:, 512:1024])

                def emit_softmax_front(t):
                    # DVE maxes + combine for step t (right after its mms)
                    st_ = state[t]
                    nm2 = tiny.tile([128, 2], F32, tag="nm2")
                    nmg = tiny.tile([128, 1], F32, tag="nmg")
                    st_["nmg"] = nmg
                    nc.vector.reduce_max(out=nm2[:, 0:1], in_=st_["s0"][:],
                                         axis=X, negate=True)
                    nc.vector.reduce_max(out=nm2[:, 1:2], in_=st_["s1"][:],
                                         axis=X, negate=True)
                    nc.vector.tensor_tensor(out=nmg[:], in0=nm2[:, 0:1],
                                            in1=nm2[:, 1:2], op=MIN)

                def emit_exp(t):
                    st_ = state[t]
                    p = p_pool.tile([128, S], FP16, tag="p")
                    den2 = tiny.tile([128, 2], F32, tag="den2")
                    st_["p"], st_["den2"] = p, den2
                    for sh in range(2):
                        nc.scalar.activation(
                            p[:, sh * HS:(sh + 1) * HS], st_["s%d" % sh][:],
                            EXP, bias=st_["nmg"][:], scale=1.0,
                            accum_out=den2[:, sh:sh + 1],
                        )
                        st_["s%d" % sh] = None

                def emit_den_recip(t):
                    # den = den0 + den1, then den <- 1/den, both on the idle
                    # Pool engine ([128,1] ops only - Pool's fp16 bulk ops
                    # are ~17ns/elem and must never touch wide data)
                    st_ = state[t]
                    den = tiny.tile([128, 1], F32, tag="den")
                    scr1 = tiny.tile([128, 1], F32, tag="scr1")
                    st_["invden"] = den
                    nc.gpsimd.tensor_tensor(out=den[:], in0=st_["den2"][:, 0:1],
                                            in1=st_["den2"][:, 1:2], op=ADD)
                    nc.gpsimd.normalize_recip(scr1[:], st_["den2"][:, 0:1],
                                              den[:])

                def emit_norm_transpose(t):
                    st_ = state[t]
                    pn = pn_pool.tile([128, S], FP16, tag="pn")
                    nc.vector.tensor_scalar(pn[:], st_["p"][:],
                                            st_["invden"][:], None, MUL)
                    st_["p"] = None
                    pt = pt_pool.tile([128, ST, 128], FP16, tag="pt")
                    st_["pt"] = pt
                    oscr = tiny.tile([128, 2], FP16, tag="oscr")
                    nc.sync.dma_start(oscr[:], pn[:, 0:2])
                    nc.sync.dma_start_transpose(pt[:], pn[:])
                    st_["pn"] = None

                out_fifo = []
                for t in range(NT):
                    qt, h = divmod(t, H)
                    state[t] = {}
                    # DVE order: normalize(t-2) first so its transpose chain
                    # releases early, then this step's maxes.
                    if t - 2 >= 0:
                        emit_norm_transpose(t - 2)
                    emit_scores_half(t, 0)
                    if t - LAG >= 0:
                        emit_ctx(t - LAG)
                        tl = t - LAG
                        qtl, hl = divmod(tl, H)
                        if hl == 3:
                            out_fifo.append((qtl, 0))
                        elif hl == 7:
                            out_fifo.append((qtl, 1))
                    emit_scores_half(t, 1)
                    if out_fifo:
                        emit_out(*out_fifo.pop(0))
                    emit_softmax_front(t)
                    if t - 1 >= 0:
                        emit_exp(t - 1)
                        emit_den_recip(t - 1)
                    if t - 2 >= 0:
                        state.pop(t - 2 - LAG, None)

                # ----- drain: finish softmax chain + ctx + out tail -------
                emit_exp(NT - 1)
                emit_den_recip(NT - 1)
                for t in (NT - 2, NT - 1):
                    emit_norm_transpose(t)
                for t in range(NT - LAG, NT):
                    emit_ctx(t)
                    qtl, hl = divmod(t, H)
                    if hl == 3:
                        out_fifo.append((qtl, 0))
                    elif hl == 7:
                        out_fifo.append((qtl, 1))
                for job in out_fifo:
                    emit_out(*job)

    nc.compile()
    return nc


def prep_inputs(x, Wq, Wk, Wv, Wo):
    scale = np.float32(1.0 / np.sqrt(O))

    def perm(w):  # (T*128, N) -> (128, T*N): partition-major tiles
        t = w.shape[0] // 128
        return np.ascontiguousarray(
            w.reshape(t, 128, -1).transpose(1, 0, 2).reshape(128, -1))

    wq = np.stack([perm(Wq[h].astype(np.float32) * scale) for h in range(H)])
    wk = perm(Wk.astype(np.float32))
    wv = perm(Wv.astype(np.float32))
    wo = perm(Wo.astype(np.float16))

    in_maps = []
    xts = {}
    for b in range(B):
        xts[b] = np.ascontiguousarray(x[b].T.astype(np.float32))  # (E, S)
    for c in range(NCORES):
        b, half = divmod(c, 2)
        xt = xts[b]
        if half == 1:
            # rotate so this core's query half occupies columns [0, SQ)
            xt = np.ascontiguousarray(np.roll(xt, SQ, axis=1))
        in_maps.append({
            "xkv": xt, "wq": wq, "wk": wk, "wv": wv, "wo": wo,
        })
    return in_maps


def kernel(x, Wq, Wk, Wv, Wo):
    global _compiled
    from concourse.bass_utils import run_bass_kernel_spmd

    x = np.asarray(x, dtype=np.float32)
    Wq = np.asarray(Wq, dtype=np.float32)
    Wk = np.asarray(Wk, dtype=np.float32)
    Wv = np.asarray(Wv, dtype=np.float32)
    Wo = np.asarray(Wo, dtype=np.float32)

    if _compiled is None:
        _compiled = _build()
    nc = _compiled

    in_maps = prep_inputs(x, Wq, Wk, Wv, Wo)

    res = run_bass_kernel_spmd(nc, in_maps, core_ids=list(range(NCORES)))

    out = np.empty((B, S, E), dtype=np.float32)
    for c in range(NCORES):
        b, half = divmod(c, 2)
        out[b, half * SQ:(half + 1) * SQ, :] = res.results[c]["out"]
    return out


# revision 15
# speedup vs baseline: 1.8158x; 1.8158x over previous
"""Multi-head self-attention on 8 TRN2 NeuronCores.

Problem: x(4,2048,1024), Wq(8,1024,128), Wk/Wv(1024,128), Wo(1024,1024) fp32.
out = softmax(Q K^T / sqrt(128)) V -> concat heads -> @ Wo.

Sharding: (batch, query-half) across 8 cores - core c handles batch c//2,
query rows [(c%2)*1024, (c%2)*1024+1024). K/V cover the full sequence of the
batch, so each core computes them locally from its x slice; no collectives.

Dataflow (v2): q-tile OUTER, head INNER so the out-projection (which needs all
heads of a q-tile) pipelines into the main loop instead of being a serial
tail. Per (qt, h) step:
  PE   : scores s0|s1 (f32r, 4x512-col) + lagged ctx (16 fp16 128-col mms
         against v_sb) + lagged out-projection mms (fp16 512-col)
  DVE  : reduce_max(negate) per score half, min-combine -> -gmax, reciprocal
  ACT  : exp per half (bias=-gmax, fp16 out, accum_out -> den halves),
         ctx-ring batch copies, out copies, small normalize slice
  Pool : den = den0+den1, bulk of the normalize (tensor_scalar by 1/den)
  DMA  : P^T via the xbar dma_start_transpose ([16,128]-tile hw transpose,
         SBUF->SBUF fp16) - replaces the PE transposes AND the PSUM->SBUF
         wave copies of the v1 kernel, freeing ~1.7us/step of PE+DVE time.
The normalization is a per-partition tensor_scalar on the q-part P (q on
partitions), which kills v1's replicate+PE-transpose+broadcast machinery.

PSUM (8 banks): 3x scores [128,1024] (6) + ctx ring [128,4,128] (1 bank,
slot=h%4) + out ring [128,512] (1 bank, ec-serialized accumulation over h).

Numerics: scores need ~12+ mantissa bits -> f32r end-to-end for x/Wq/Wk
(1 cycle/row at >=256-wide moving). P/V/ctx/Wo run fp16. exp against the
global row max: P in (0,1], den ~ O(1) in fp32 accumulators.
"""
import numpy as np

B, S, E, H, O = 4, 2048, 1024, 8, 128
SQ = S // 2          # query rows per core
NCORES = 8
ET = E // 128        # 8 e-tiles
ST = S // 128        # 16 s-tiles
QT = SQ // 128       # 8 q-tiles
HS = S // 2          # 1024-wide score half

LAG = 5              # steps between scores(t) and ctx(t) consumption
NORM_ACT = 128       # normalize slice handled by ACT (rest: DVE)

_compiled = None     # cache so repeated kernel() calls skip rebuild


def _build():
    import concourse.bass as bass
    import concourse.mybir as mybir
    import concourse.tile as tile
    from concourse import bacc

    F32 = mybir.dt.float32
    F32R = mybir.dt.float32r
    FP16 = mybir.dt.float16
    PS = bass.MemorySpace.PSUM
    EXP = mybir.ActivationFunctionType.Exp
    MIN = mybir.AluOpType.min
    ADD = mybir.AluOpType.add
    MUL = mybir.AluOpType.mult
    X = mybir.AxisListType.X
    IDN = mybir.ActivationFunctionType.Identity

    nc = bacc.Bacc("TRN2", target_bir_lowering=False, debug=False,
                   enable_asserts=True)

    # xkv columns pre-permuted per core so its query half is always columns
    # [0, SQ) - attention is permutation-invariant over the key axis.
    d_xkv = nc.dram_tensor("xkv", (E, S), F32R, kind="ExternalInput").ap()
    d_wq = nc.dram_tensor("wq", (H, 128, ET * O), F32R, kind="ExternalInput").ap()
    d_wk = nc.dram_tensor("wk", (128, ET * O), F32R, kind="ExternalInput").ap()
    d_wv = nc.dram_tensor("wv", (128, ET * O), F32R, kind="ExternalInput").ap()
    d_wo = nc.dram_tensor("wo", (128, H * E), FP16, kind="ExternalInput").ap()
    d_out = nc.dram_tensor("out", (SQ, E), F32, kind="ExternalOutput").ap()

    with tile.TileContext(nc) as tc:
        with (
            tc.tile_pool(name="persist", bufs=1) as persist,
            tc.tile_pool(name="tiny", bufs=24) as tiny,
        ):
            wo_sb = persist.tile([128, H, E], FP16, tag="wo")
            kt = persist.tile([128, S], F32R, tag="kt")
            qt_sb = persist.tile([128, H, SQ], F32R, tag="qt")
            v_sb = persist.tile([128, ST, O], FP16, tag="v")

            # ---------------- prologue: K^T, V, Q^T projections ----------
            with tc.tile_pool(name="xp", bufs=1) as xp:
                wk = xp.tile([128, ET, O], F32R, tag="wk")
                wv = xp.tile([128, ET, O], F32R, tag="wv")
                xkv = xp.tile([128, ET, S], F32R, tag="xkv")
                wq = xp.tile([128, H, ET, O], F32R, tag="wq")
                vt_sb = xp.tile([128, S], FP16, tag="vtsb")

                # x e-tiles split over the two HWDGE queues; weights on the
                # gpsimd SWDGE queue so x saturates the aggregate early.
                for e in range(0, ET, 2):
                    nc.sync.dma_start(
                        xkv[:, e, :], d_xkv[e * 128:(e + 1) * 128, :])
                    nc.scalar.dma_start(
                        xkv[:, e + 1, :],
                        d_xkv[(e + 1) * 128:(e + 2) * 128, :])
                nc.sync.dma_start(wk[:].rearrange("p t o -> p (t o)"), d_wk)
                nc.scalar.dma_start(wv[:].rearrange("p t o -> p (t o)"), d_wv)
                for h in range(H):
                    nc.gpsimd.dma_start(
                        wq[:, h, :, :].rearrange("p t o -> p (t o)"), d_wq[h])
                nc.gpsimd.dma_start(
                    wo_sb[:].rearrange("p h e -> p (h e)"), d_wo)

                # K^T and V^T share one PSUM scope (4+4 banks); e-major so
                # the PE starts on the first arriving e-tile.
                with (
                    tc.tile_pool(name="ktp", bufs=1, space=PS) as ktp,
                    tc.tile_pool(name="vtp", bufs=1, space=PS) as vtp,
                ):
                    kt_ps = ktp.tile([128, S], F32, tag="kt")
                    vt_ps = vtp.tile([128, S], F32, tag="vt")
                    for e in range(ET):
                        for c in range(4):
                            nc.tensor.matmul(
                                kt_ps[:, c * 512:(c + 1) * 512],
                                wk[:, e, :],
                                xkv[:, e, c * 512:(c + 1) * 512],
                                start=(e == 0), stop=(e == ET - 1),
                            )
                        for c in range(4):
                            nc.tensor.matmul(
                                vt_ps[:, c * 512:(c + 1) * 512],
                                wv[:, e, :],
                                xkv[:, e, c * 512:(c + 1) * 512],
                                start=(e == 0), stop=(e == ET - 1),
                            )
                    nc.scalar.copy(kt[:], kt_ps[:])
                    nc.scalar.copy(vt_sb[:], vt_ps[:])
                # V^T (O,S) -> v_sb[kp, st, o] = V[st*128+kp, o] via the
                # hw xbar transpose (replaces 16 PE transposes + copies).
                ord_scr = xp.tile([128, 2], FP16, tag="ordscr")
                nc.sync.dma_start(ord_scr[:], vt_sb[:, 0:2])
                nc.sync.dma_start_transpose(v_sb[:], vt_sb[:])

                # Q^T per head (gated on wq[h] + all of x)
                with tc.tile_pool(name="qp", bufs=3, space=PS) as qp:
                    for h in range(H):
                        q_ps = qp.tile([128, SQ], F32, tag="qtps")
                        for e in range(ET):
                            for c in range(SQ // 512):
                                nc.tensor.matmul(
                                    q_ps[:, c * 512:(c + 1) * 512],
                                    wq[:, h, e, :],
                                    xkv[:, e, c * 512:(c + 1) * 512],
                                    start=(e == 0), stop=(e == ET - 1),
                                )
                        nc.scalar.copy(qt_sb[:, h, :], q_ps[:])

            # ---------------- main: qt outer, head inner ------------------
            with (
                tc.tile_pool(name="p_pool", bufs=4) as p_pool,
                tc.tile_pool(name="pn_pool", bufs=4) as pn_pool,
                tc.tile_pool(name="pt_pool", bufs=LAG + 2) as pt_pool,
                tc.tile_pool(name="cx_pool", bufs=3) as cx_pool,
                tc.tile_pool(name="o_sbp", bufs=2) as o_sbp,
                tc.tile_pool(name="sc_ps", bufs=3, space=PS) as sc_psp,
                tc.tile_pool(name="ct_ps", bufs=1, space=PS) as ct_psp,
                tc.tile_pool(name="o_ps", bufs=1, space=PS) as o_psp,
            ):
                ct_ring = ct_psp.tile([128, 4, 128], F32, tag="ct")
                o_ring = o_psp.tile([128, 512], F32, tag="op")

                NT = H * QT
                state = {}   # per-step tiles: s0, s1, p, pn, pt, nm2, den2..
                ctxs = {}    # qt -> ctxs_sb tile [128, H, 128]

                def emit_scores_half(t, sh):
                    qt, h = divmod(t, H)
                    st_ = state[t]
                    s_ps = sc_psp.tile([128, HS], F32, tag="sc")
                    st_["s%d" % sh] = s_ps
                    for c in range(2):
                        nc.tensor.matmul(
                            s_ps[:, c * 512:(c + 1) * 512],
                            qt_sb[:, h, qt * 128:(qt + 1) * 128],
                            kt[:, sh * HS + c * 512:sh * HS + (c + 1) * 512],
                            start=True, stop=True,
                        )

                def emit_ctx(t):
                    # ctx^T(o-part, 128 q cols) for step t, into ring slot
                    qt, h = divmod(t, H)
                    st_ = state[t]
                    slot = h % 4
                    for st in range(ST):
                        nc.tensor.matmul(
                            ct_ring[:, slot, :],
                            v_sb[:, st, :],
                            st_["pt"][:, st, :],
                            start=(st == 0), stop=(st == ST - 1),
                        )
                    if slot == 3:
                        # batch-copy 4 head slots into the qt's ctx tile
                        if qt not in ctxs:
                            ctxs[qt] = cx_pool.tile([128, H, 128], FP16,
                                                    tag="cx",
                                                    name="cx%d" % qt)
                        nc.scalar.copy(
                            ctxs[qt][:, h - 3:h + 1, :], ct_ring[:])
                    st_["pt"] = None  # allow pool reuse

                def emit_out(qt, part):
                    # out-projection for q-tile qt; part 0: heads 0-3 of the
                    # ec0 group; part 1: heads 4-7 + full ec1 group + copies
                    cx = ctxs[qt]
                    if part == 0:
                        for h in range(4):
                            nc.tensor.matmul(
                                o_ring[:], cx[:, h, :],
                                wo_sb[:, h, 0:512],
                                start=(h == 0), stop=False,
                            )
                        return
                    for h in range(4, H):
                        nc.tensor.matmul(
                            o_ring[:], cx[:, h, :],
                            wo_sb[:, h, 0:512],
                            start=False, stop=(h == H - 1),
                        )
                    o_sb = o_sbp.tile([128, E], F32, tag="osb")
                    nc.scalar.copy(o_sb[:, 0:512], o_ring[:])
                    nc.sync.dma_start(
                        d_out[qt * 128:(qt + 1) * 128, 0:512],
                        o_sb[:, 0:512])
                    for h in range(H):
                        nc.tensor.matmul(
                            o_ring[:], cx[:, h, :],
                            wo_sb[:, h, 512:1024],
                            start=(h == 0), stop=(h == H - 1),
                        )
                    nc.scalar.copy(o_sb[:, 512:1024], o_ring[:])
                    nc.sync.dma_start(
                        d_out[qt * 128:(qt + 1) * 128, 512:1024],
                        o_sb[:, 512:1024])

                def emit_softmax_front(t):
                    # DVE maxes + combine for step t (right after its mms)
                    st_ = state[t]
                    nm2 = tiny.tile([128, 2], F32, tag="nm2")
                    nmg = tiny.tile([128, 1], F32, tag="nmg")
                    st_["nmg"] = nmg
                    nc.vector.reduce_max(out=nm2[:, 0:1], in_=st_["s0"][:],
                                         axis=X, negate=True)
                    nc.vector.reduce_max(out=nm2[:, 1:2], in_=st_["s1"][:],
                                         axis=X, negate=True)
                    nc.vector.tensor_tensor(out=nmg[:], in0=nm2[:, 0:1],
                                            in1=nm2[:, 1:2], op=MIN)

                def emit_exp(t):
                    st_ = state[t]
                    p = p_pool.tile([128, S], FP16, tag="p")
                    den2 = tiny.tile([128, 2], F32, tag="den2")
                    st_["p"], st_["den2"] = p, den2
                    for sh in range(2):
                        nc.scalar.activation(
                            p[:, sh * HS:(sh + 1) * HS], st_["s%d" % sh][:],
                            EXP, bias=st_["nmg"][:], scale=1.0,
                            accum_out=den2[:, sh:sh + 1],
                        )
                        st_["s%d" % sh] = None

                def emit_den_recip(t):
                    # den = den0 + den1 on Pool (ONLY standard tensor_tensor
                    # there: mixing custom Q7 ops forces a ~7us library
                    # reload per op). recip on DVE.
                    st_ = state[t]
                    den = tiny.tile([128, 1], F32, tag="den")
                    invden = tiny.tile([128, 1], F32, tag="invden")
                    st_["invden"] = invden
                    nc.gpsimd.tensor_tensor(out=den[:], in0=st_["den2"][:, 0:1],
                                            in1=st_["den2"][:, 1:2], op=ADD)
                    nc.vector.reciprocal(invden[:], den[:])

                def emit_norm_transpose(t):
                    st_ = state[t]
                    pn = pn_pool.tile([128, S], FP16, tag="pn")
                    nc.scalar.activation(pn[:, 0:NORM_ACT], st_["p"][:, 0:NORM_ACT],
                                         IDN, bias=0.0, scale=st_["invden"][:])
                    nc.vector.tensor_scalar(pn[:, NORM_ACT:S],
                                            st_["p"][:, NORM_ACT:S],
                                            st_["invden"][:], None, MUL)
                    st_["p"] = None
                    pt = pt_pool.tile([128, ST, 128], FP16, tag="pt")
                    st_["pt"] = pt
                    oscr = tiny.tile([128, 2], FP16, tag="oscr")
                    nc.sync.dma_start(oscr[:], pn[:, 0:2])
                    nc.sync.dma_start_transpose(pt[:], pn[:])
                    st_["pn"] = None

                out_fifo = []
                for t in range(NT):
                    qt, h = divmod(t, H)
                    state[t] = {}
                    # DVE order: normalize(t-2) first so its transpose chain
                    # releases early, then this step's maxes.
                    if t - 2 >= 0:
                        emit_norm_transpose(t - 2)
                    emit_scores_half(t, 0)
                    if t - LAG >= 0:
                        emit_ctx(t - LAG)
                        tl = t - LAG
                        qtl, hl = divmod(tl, H)
                        if hl == 3:
                            out_fifo.append((qtl, 0))
                        elif hl == 7:
                            out_fifo.append((qtl, 1))
                    emit_scores_half(t, 1)
                    if out_fifo:
                        emit_out(*out_fifo.pop(0))
                    emit_softmax_front(t)
                    if t - 1 >= 0:
                        emit_exp(t - 1)
                        emit_den_recip(t - 1)
                    if t - 2 >= 0:
                        state.pop(t - 2 - LAG, None)

                # ----- drain: finish softmax chain + ctx + out tail -------
                emit_exp(NT - 1)
                emit_den_recip(NT - 1)
                for t in (NT - 2, NT - 1):
                    emit_norm_transpose(t)
                for t in range(NT - LAG, NT):
                    emit_ctx(t)
                    qtl, hl = divmod(t, H)
                    if hl == 3:
                        out_fifo.append((qtl, 0))
                    elif hl == 7:
                        out_fifo.append((qtl, 1))
                for job in out_fifo:
                    emit_out(*job)

    nc.compile()
    return nc


def prep_inputs(x, Wq, Wk, Wv, Wo):
    scale = np.float32(1.0 / np.sqrt(O))

    def perm(w):  # (T*128, N) -> (128, T*N): partition-major tiles
        t = w.shape[0] // 128
        return np.ascontiguousarray(
            w.reshape(t, 128, -1).transpose(1, 0, 2).reshape(128, -1))

    wq = np.stack([perm(Wq[h].astype(np.float32) * scale) for h in range(H)])
    wk = perm(Wk.astype(np.float32))
    wv = perm(Wv.astype(np.float32))
    wo = perm(Wo.astype(np.float16))

    in_maps = []
    xts = {}
    for b in range(B):
        xts[b] = np.ascontiguousarray(x[b].T.astype(np.float32))  # (E, S)
    for c in range(NCORES):
        b, half = divmod(c, 2)
        xt = xts[b]
        if half == 1:
            # rotate so this core's query half occupies columns [0, SQ)
            xt = np.ascontiguousarray(np.roll(xt, SQ, axis=1))
        in_maps.append({
            "xkv": xt, "wq": wq, "wk": wk, "wv": wv, "wo": wo,
        })
    return in_maps


def kernel(x, Wq, Wk, Wv, Wo):
    global _compiled
    from concourse.bass_utils import run_bass_kernel_spmd

    x = np.asarray(x, dtype=np.float32)
    Wq = np.asarray(Wq, dtype=np.float32)
    Wk = np.asarray(Wk, dtype=np.float32)
    Wv = np.asarray(Wv, dtype=np.float32)
    Wo = np.asarray(Wo, dtype=np.float32)

    if _compiled is None:
        _compiled = _build()
    nc = _compiled

    in_maps = prep_inputs(x, Wq, Wk, Wv, Wo)

    res = run_bass_kernel_spmd(nc, in_maps, core_ids=list(range(NCORES)))

    out = np.empty((B, S, E), dtype=np.float32)
    for c in range(NCORES):
        b, half = divmod(c, 2)
        out[b, half * SQ:(half + 1) * SQ, :] = res.results[c]["out"]
    return out
